# revision 1
# baseline (speedup 1.0000x reference)
"""BRU (bistable recurrent unit) cell kernel for 8 Trainium2 NeuronCores.

Hardcoded problem: B=64, T=512, D=1024, U=1024, fp32.

Sharding: 8 cores = 4 batch-groups (16 batches each) x 2 unit-groups
(512 units each).  Per core the three input projections
    projT[u, token] = K[d,u].T @ xT[d, token],   token = b*512 + t
run on the PE in fp16 with a 3-term split for fp32-grade accuracy:
    x @ K  ~=  A@K1 + e@K1 + (A*2^-12)@(K2*2^12)
with A = fp16(x), e = fp16(x - A), K1 = fp16(K), K2 = K - K1 (scaled by
2^12 into fp16 normal range; the power-of-two scales cancel exactly).
All terms run at 1 cycle/row on the PE and accumulate in fp32 PSUM.

The 512-step recurrence is elementwise with u on partitions, split into
two batch-group chains that are software-pipelined by emission order
(engines dispatch in-order).  Sigmoid is re-expressed via tanh so each
group needs only 2 activations per step (one merged t1/tau tanh + one
hh tanh), and 7 fused DVE ops.  Projections are chunked TC steps at a
time (double-buffered) so the PE runs ahead of the recurrence;
PSUM->SBUF copies ride on the Scalar engine, dripped between scan
steps, and fold in the bias when nonzero.
"""

import os

import numpy as np

B, T, D, U = 64, 512, 1024, 1024
NCORES = 8
NBG = 4  # batch groups
NUG = 2  # unit groups
BL = B // NBG  # 16 batches per core
UHALF = U // NUG  # 512 units per core
UH = UHALF // 128  # 4 u-chunks

_CACHE: dict = {}


def _build(T_, TC, use_memory, use_bias):
    """Build and compile the per-core Bass program."""
    import concourse.mybir as mybir
    from concourse import bacc
    from concourse.tile import TileContext

    f32 = mybir.dt.float32
    f16 = mybir.dt.float16
    Alu = mybir.AluOpType
    Act = mybir.ActivationFunctionType

    NTOK = BL * T_
    NCH = T_ // TC
    DC = D // 128  # 8 d-chunks

    nc = bacc.Bacc("TRN2", target_bir_lowering=False, debug=False)

    xA = nc.dram_tensor("xA", [D, NTOK], f16, kind="ExternalInput").ap()
    xE = nc.dram_tensor("xE", [D, NTOK], f16, kind="ExternalInput").ap()
    xS = nc.dram_tensor("xS", [D, NTOK], f16, kind="ExternalInput").ap()
    k1 = {}
    k2 = {}
    for g in "zrh":
        k1[g] = nc.dram_tensor(f"k1{g}", [D, UHALF], f16, kind="ExternalInput").ap()
        k2[g] = nc.dram_tensor(f"k2{g}", [D, UHALF], f16, kind="ExternalInput").ap()
    if use_memory:
        mzb = nc.dram_tensor("mzb", [128, UH, BL], f32, kind="ExternalInput").ap()
        mrb = nc.dram_tensor("mrb", [128, UH, BL], f32, kind="ExternalInput").ap()
    if use_bias:
        bts = {
            g: nc.dram_tensor(f"bt{g}", [128, UH], f32, kind="ExternalInput").ap()
            for g in "zrh"
        }
    outT = nc.dram_tensor("outT", [UHALF, NTOK], f32, kind="ExternalOutput").ap()

    xA_r = xA.rearrange("(dc p) (b t) -> dc p b t", dc=DC, b=BL)
    xE_r = xE.rearrange("(dc p) (b t) -> dc p b t", dc=DC, b=BL)
    xS_r = xS.rearrange("(dc p) (b t) -> dc p b t", dc=DC, b=BL)
    outT_r = outT.rearrange("(uh p) (b t) -> uh p b t", uh=UH, b=BL)

    # Uniform chunk schedule.  (Tapered variants — short chunks at the
    # start and/or end — were measured no better: the scan drain after the
    # last matmul is set by the scan's per-chunk rate, not chunk sizes.)
    chunks = [TC] * (T_ // TC)
    assert sum(chunks) == T_, (chunks, T_)

    with TileContext(nc) as tc:
        with (
            tc.tile_pool(name="weights", bufs=1) as wpool,
            tc.tile_pool(name="xin", bufs=2) as xpool,
            tc.tile_pool(name="proj", bufs=2) as ppool,
            tc.tile_pool(name="hout", bufs=3) as hpool,
            tc.tile_pool(name="tmp", bufs=12) as spool,
            tc.tile_pool(name="misc", bufs=1) as mpool,
            tc.tile_pool(name="psum", bufs=8, space="PSUM") as qpool,
        ):
            # Startup order: z-gate weights, then the first x chunk, then
            # the remaining weights, so the PE's first PSUM group can start
            # as early as possible.
            TC0 = chunks[0]
            w1 = {}
            w2 = {}
            for g in "zrh":
                w1[g] = wpool.tile([128, DC, UHALF], f16, tag=f"w1{g}", name=f"w1{g}")
                w2[g] = wpool.tile([128, DC, UHALF], f16, tag=f"w2{g}", name=f"w2{g}")
            nc.sync.dma_start(
                w1["z"][:, :, :], k1["z"].rearrange("(dc p) u -> p dc u", p=128)
            )
            xa = xpool.tile([128, DC, BL, TC], f16, tag="xa", name="xa_0")
            xe = xpool.tile([128, DC, BL, TC], f16, tag="xe", name="xe_0")
            xs = xpool.tile([128, DC, BL, TC], f16, tag="xs", name="xs_0")
            for dc in range(DC):
                nc.sync.dma_start(xa[:, dc, :, :TC0], xA_r[dc, :, :, 0:TC0])
                nc.sync.dma_start(xe[:, dc, :, :TC0], xE_r[dc, :, :, 0:TC0])
            nc.sync.dma_start(
                w2["z"][:, :, :], k2["z"].rearrange("(dc p) u -> p dc u", p=128)
            )
            for dc in range(DC):
                nc.sync.dma_start(xs[:, dc, :, :TC0], xS_r[dc, :, :, 0:TC0])
            first_x = (xa, xe, xs)
            for g in "rh":
                nc.sync.dma_start(
                    w1[g][:, :, :], k1[g].rearrange("(dc p) u -> p dc u", p=128)
                )
                nc.sync.dma_start(
                    w2[g][:, :, :], k2[g].rearrange("(dc p) u -> p dc u", p=128)
                )
            if use_memory:
                # host passes mzb = 0.25*m_z, mrb = 0.5*m_r broadcasts
                mz4_t = mpool.tile([128, UH, BL], f32, tag="mz4", name="mz4")
                mr2_t = mpool.tile([128, UH, BL], f32, tag="mr2", name="mr2")
                nc.sync.dma_start(mz4_t[:, :, :], mzb[:, :, :])
                nc.sync.dma_start(mr2_t[:, :, :], mrb[:, :, :])
            if use_bias:
                b_t = {}
                for g in "zrh":
                    b_t[g] = mpool.tile([128, UH], f32, tag=f"b{g}", name=f"b{g}")
                    nc.sync.dma_start(b_t[g][:, :], bts[g][:, :])

            h0 = []
            for gi in range(2):
                h0g = mpool.tile([128, UH, BL // 2], f32, tag=f"h0{gi}", name=f"h0{gi}")
                nc.gpsimd.memset(h0g[:, :, :], 0.0)
                h0.append(h0g)

            # ---------------------------------------------------------
            # Software-pipelined scan over two batch-group chains.
            #
            # Math (per step, with carried state v = 2h):
            #   t1  = tanh(h*m_r + xr)
            #   tau = tanh(0.5*(h*m_z + xz)) so  1-z = 0.5*(1-tau)
            #   hh  = tanh(xh + (t1+1)*h)
            #   v'  = 2h' = (v/2 + hh) + tau*(v/2 - hh)
            # The host folds 0.5 into the z-gate weights/bias and halves the
            # output, so the kernel stores v.  t1 and tau come from ONE
            # merged Tanh per group (the Scalar engine dispatches serially,
            # ~270ns per instruction, so activation count dominates).
            # ---------------------------------------------------------
            GROUPS = ((0, BL // 2), (BL // 2, BL))
            HB = BL // 2

            v0t = []
            for gi in range(2):
                vg = mpool.tile([128, UH, HB], f32, tag=f"v0{gi}", name=f"v0{gi}")
                nc.gpsimd.memset(vg[:, :, :], 0.0)
                v0t.append(vg)

            def tmp(tag, gi, shape=None):
                return spool.tile(shape or [128, UH, HB], f32, tag=f"{tag}{gi}",
                                  name=f"{tag}{gi}")

            state = [dict(), dict()]

            def stage_F(gi, v, pz, pr, trel):
                """stg[0] = t1in = h*m_r + xr;  stg[1] = 0.5*zin = h*mz/2 + xz/2.
                (xz/2 is pre-folded into the z projection host-side.)"""
                s = state[gi] = {}
                s["stg"] = tmp("stg", gi, [128, 2, UH, HB])
                b0, b1 = GROUPS[gi]
                xr_t = pr[:, :, b0:b1, trel]
                xzh_t = pz[:, :, b0:b1, trel]
                if use_memory:
                    hm_r = tmp("hmr", gi)
                    hm_z = tmp("hmz", gi)
                    nc.vector.tensor_mul(hm_r[:, :, :], v, mr2_t[:, :, b0:b1])
                    nc.vector.tensor_add(s["stg"][:, 0, :, :], hm_r[:, :, :], xr_t)
                    nc.vector.tensor_mul(hm_z[:, :, :], v, mz4_t[:, :, b0:b1])
                    nc.vector.tensor_add(s["stg"][:, 1, :, :], hm_z[:, :, :], xzh_t)
                else:
                    # t1in = v*0.5 + xr ; tau_in = v*0.25 + xz/2
                    nc.vector.scalar_tensor_tensor(
                        s["stg"][:, 0, :, :], v, 0.5, xr_t, Alu.mult, Alu.add
                    )
                    nc.vector.scalar_tensor_tensor(
                        s["stg"][:, 1, :, :], v, 0.25, xzh_t, Alu.mult, Alu.add
                    )
                s["v"] = v

            def stage_X(gi):
                s = state[gi]
                s["sto"] = tmp("sto", gi, [128, 2, UH, HB])
                nc.scalar.activation(
                    s["sto"][:, :, :, :], s["stg"][:, :, :, :], Act.Tanh
                )

            def stage_M(gi, ph, trel):
                s = state[gi]
                b0, b1 = GROUPS[gi]
                xh_t = ph[:, :, b0:b1, trel]
                w = tmp("w", gi)
                # w = (t1 + 1) * v
                nc.vector.scalar_tensor_tensor(
                    w[:, :, :], s["sto"][:, 0, :, :], 1.0, s["v"], Alu.add, Alu.mult
                )
                # hhin = 0.5*w + xh = (t1+1)*h + xh
                s["hin"] = tmp("hin", gi)
                nc.vector.scalar_tensor_tensor(
                    s["hin"][:, :, :], w[:, :, :], 0.5, xh_t, Alu.mult, Alu.add
                )
                # pre-compute the hh-independent half of the blend:
                # w2 = (1+tau)*v, so v' = 0.5*w2 - (tau-1)*hh
                tau = s["sto"][:, 1, :, :]
                s["w2"] = tmp("w2", gi)
                nc.vector.scalar_tensor_tensor(
                    s["w2"][:, :, :], tau, 1.0, s["v"], Alu.add, Alu.mult
                )

            def stage_H(gi):
                s = state[gi]
                s["hh"] = tmp("hh", gi)
                nc.scalar.activation(s["hh"][:, :, :], s["hin"][:, :, :], Act.Tanh)

            def stage_B(gi, hch_g, trel):
                # v' = 0.5*w2 - (tau-1)*hh
                s = state[gi]
                hh = s["hh"][:, :, :]
                r1 = tmp("r1", gi)
                nc.vector.scalar_tensor_tensor(
                    r1[:, :, :], s["sto"][:, 1, :, :], 1.0, hh,
                    Alu.subtract, Alu.mult,
                )
                nc.vector.scalar_tensor_tensor(
                    hch_g[:, :, :, trel], s["w2"][:, :, :], 0.5, r1[:, :, :],
                    Alu.mult, Alu.subtract,
                )

            def emit_matmuls(c, TCc, xa, xe, xs):
                projs = {}
                copies = []
                for g in "zrh":
                    pg = ppool.tile(
                        [128, UH, BL, TC], f32, tag=f"p{g}", name=f"p{g}_{c}"
                    )
                    projs[g] = pg
                    for uh in range(UH):
                        us = slice(uh * 128, (uh + 1) * 128)
                        ps = qpool.tile([128, BL, TC], f32, tag="ps")
                        for dc in range(DC):
                            nc.tensor.matmul(
                                ps[:, :, :TCc], w1[g][:, dc, us], xa[:, dc, :, :TCc],
                                start=(dc == 0), stop=False,
                            )
                            nc.tensor.matmul(
                                ps[:, :, :TCc], w1[g][:, dc, us], xe[:, dc, :, :TCc],
                                start=False, stop=False,
                            )
                        for dc in range(DC):
                            nc.tensor.matmul(
                                ps[:, :, :TCc], w2[g][:, dc, us], xs[:, dc, :, :TCc],
                                start=False, stop=(dc == DC - 1),
                            )

                        def mkcopy(pg=pg, uh=uh, ps=ps, g=g):
                            def do():
                                if use_bias:
                                    nc.scalar.activation(
                                        pg[:, uh, :, :TCc], ps[:, :, :TCc],
                                        Act.Identity, bias=b_t[g][:, uh : uh + 1],
                                    )
                                else:
                                    nc.scalar.activation(
                                        pg[:, uh, :, :TCc], ps[:, :, :TCc],
                                        Act.Identity,
                                    )
                            return do

                        copies.append(mkcopy())
                return projs, copies

            def emit_scan(sc, TCsc, projs, prev_v, prev_tc, pending):
                pz, pr, ph = projs["z"], projs["r"], projs["h"]
                hch = [
                    hpool.tile([128, UH, HB, TC], f32, tag=f"hch{gi}",
                               name=f"hch{gi}_{sc}")
                    for gi in range(2)
                ]

                def v_of(gi, trel):
                    if trel == 0:
                        if sc == 0:
                            return v0t[gi][:, :, :]
                        return prev_v[gi][:, :, :, prev_tc - 1]
                    return hch[gi][:, :, :, trel - 1]

                ncopies = len(pending)
                emitted = 0

                def drip(trel):
                    nonlocal emitted
                    want = ((trel + 1) * ncopies) // max(TCsc - 1, 1)
                    while emitted < min(want, ncopies):
                        pending[emitted]()
                        emitted += 1

                def mids(trel):
                    stage_M(0, ph, trel)
                    stage_X(1)
                    stage_M(1, ph, trel)
                    stage_H(0)
                    stage_H(1)

                # prologue (trel = 0)
                stage_F(0, v_of(0, 0), pz, pr, 0)
                stage_X(0)
                stage_F(1, v_of(1, 0), pz, pr, 0)
                mids(0)
                for trel in range(1, TCsc):
                    cur0, cur1 = state[0], state[1]
                    stage_B(0, hch[0], trel - 1)
                    stage_F(0, v_of(0, trel), pz, pr, trel)
                    new0 = state[0]
                    stage_X(0)
                    state[0], state[1] = cur0, cur1
                    stage_B(1, hch[1], trel - 1)
                    state[0] = new0
                    stage_F(1, v_of(1, trel), pz, pr, trel)
                    mids(trel)
                    drip(trel - 1)
                stage_B(0, hch[0], TCsc - 1)
                stage_B(1, hch[1], TCsc - 1)
                while emitted < ncopies:
                    pending[emitted]()
                    emitted += 1
                return hch

            # main pipeline over chunks
            prev_v = None
            prev_tc = None
            prev_projs = None
            t0 = 0
            t0s = []
            for c, TCc in enumerate(chunks):
                if c == 0:
                    xa, xe, xs = first_x
                else:
                    xa = xpool.tile([128, DC, BL, TC], f16, tag="xa", name=f"xa_{c}")
                    xe = xpool.tile([128, DC, BL, TC], f16, tag="xe", name=f"xe_{c}")
                    xs = xpool.tile([128, DC, BL, TC], f16, tag="xs", name=f"xs_{c}")
                    for dc in range(DC):
                        nc.sync.dma_start(
                            xa[:, dc, :, :TCc], xA_r[dc, :, :, t0 : t0 + TCc]
                        )
                        nc.sync.dma_start(
                            xe[:, dc, :, :TCc], xE_r[dc, :, :, t0 : t0 + TCc]
                        )
                        nc.sync.dma_start(
                            xs[:, dc, :, :TCc], xS_r[dc, :, :, t0 : t0 + TCc]
                        )
                projs, copies = emit_matmuls(c, TCc, xa, xe, xs)
                if c == 0:
                    for do in copies:
                        do()
                else:
                    sc = c - 1
                    TCsc = chunks[sc]
                    hch = emit_scan(sc, TCsc, prev_projs, prev_v, prev_tc, copies)
                    for uh in range(UH):
                        for gi, (b0, b1) in enumerate(GROUPS):
                            nc.sync.dma_start(
                                outT_r[uh, :, b0:b1, t0s[sc] : t0s[sc] + TCsc],
                                hch[gi][:, uh, :, :TCsc],
                            )
                    prev_v = hch
                    prev_tc = TCsc
                prev_projs = projs
                t0s.append(t0)
                t0 += TCc
            sc = len(chunks) - 1
            TCsc = chunks[sc]
            hch = emit_scan(sc, TCsc, prev_projs, prev_v, prev_tc, [])
            for uh in range(UH):
                for gi, (b0, b1) in enumerate(GROUPS):
                    nc.sync.dma_start(
                        outT_r[uh, :, b0:b1, t0s[sc] : t0s[sc] + TCsc],
                        hch[gi][:, uh, :, :TCsc],
                    )

    nc.compile()
    return nc


def _get_nc(T_, TC, use_memory, use_bias):
    key = (T_, TC, use_memory, use_bias)
    if key not in _CACHE:
        _CACHE[key] = _build(T_, TC, use_memory, use_bias)
    return _CACHE[key]


def kernel(
    x,
    kernel_z,
    kernel_r,
    kernel_h,
    memory_z,
    memory_r,
    bias_z,
    bias_r,
    bias_h,
):
    from concourse import bass_utils

    x = np.asarray(x, dtype=np.float32)
    Ks = {
        "z": np.asarray(kernel_z, dtype=np.float32),
        "r": np.asarray(kernel_r, dtype=np.float32),
        "h": np.asarray(kernel_h, dtype=np.float32),
    }
    mem = {
        "z": np.asarray(memory_z, dtype=np.float32),
        "r": np.asarray(memory_r, dtype=np.float32),
    }
    bias = {
        "z": np.asarray(bias_z, dtype=np.float32),
        "r": np.asarray(bias_r, dtype=np.float32),
        "h": np.asarray(bias_h, dtype=np.float32),
    }

    B_, T_, D_ = x.shape
    assert (B_, D_) == (B, D), (x.shape,)
    TC = int(os.environ.get("BRU_TC", "32"))

    use_memory = not all(np.all(m == 1.0) for m in mem.values())
    use_bias = not all(np.all(b == 0.0) for b in bias.values())

    nc = _get_nc(T_, TC, use_memory, use_bias)

    # Split weights once (shared across cores).  The z-gate weights/bias are
    # pre-halved: the kernel computes tau = tanh(0.5*zin) instead of
    # sigmoid(zin).
    w1_full = {}
    w2_full = {}
    for g, K in Ks.items():
        if g == "z":
            K = K * np.float32(0.5)
        K1 = K.astype(np.float16)
        K2s = ((K - K1.astype(np.float32)) * 4096.0).astype(np.float16)
        w1_full[g] = K1
        w2_full[g] = K2s

    in_maps = []
    for c in range(NCORES):
        bg, ug = divmod(c, NUG)
        xc = x[bg * BL : (bg + 1) * BL].reshape(BL * T_, D)
        xcT = np.ascontiguousarray(xc.T)  # [D, NTOK] fp32
        A = xcT.astype(np.float16)
        e = (xcT - A.astype(np.float32)).astype(np.float16)
        As = (A.astype(np.float32) * (2.0 ** -12)).astype(np.float16)
        us = slice(ug * UHALF, (ug + 1) * UHALF)
        m = {"xA": A, "xE": e, "xS": As}
        for g in "zrh":
            m[f"k1{g}"] = np.ascontiguousarray(w1_full[g][:, us])
            m[f"k2{g}"] = np.ascontiguousarray(w2_full[g][:, us])
        if use_memory:
            # element (p, uh, b) = mem[ug*UHALF + uh*128 + p], pre-scaled
            for name, v, sc_ in (
                ("mzb", mem["z"], 0.25),
                ("mrb", mem["r"], 0.5),
            ):
                mv = (v[us] * np.float32(sc_)).reshape(UH, 128).T  # [128, UH]
                m[name] = np.ascontiguousarray(
                    np.broadcast_to(mv[:, :, None], (128, UH, BL))
                )
        if use_bias:
            for g in "zrh":
                bv = bias[g][us]
                if g == "z":
                    bv = bv * np.float32(0.5)
                m[f"bt{g}"] = np.ascontiguousarray(bv.reshape(UH, 128).T)
        in_maps.append(m)

    res = bass_utils.run_bass_kernel_spmd(nc, in_maps, core_ids=list(range(NCORES)))

    out = np.empty((B, T_, U), dtype=np.float32)
    for c in range(NCORES):
        bg, ug = divmod(c, NUG)
        oT = res.results[c]["outT"]  # [UHALF, BL*T_] holding v = 2h
        out[bg * BL : (bg + 1) * BL, :, ug * UHALF : (ug + 1) * UHALF] = (
            oT.reshape(UHALF, BL, T_).transpose(1, 2, 0)
        )
    out *= np.float32(0.5)
    return out

